# revision 2
# baseline (speedup 1.0000x reference)
"""Trainium2 Bass kernel for nn_Linear_regression (quadratic regression dot).

out0 = dot(w_lin, x) + dot(w_quad, x*x) + w[2W]
out1 = x[W//2] - out0

Strategy: shard x / w_lin / w_quad along W across 8 cores. The streams are
cast to bf16 on the host during packing (the rel-err gate is 2e-2; bf16
rounding contributes ~3e-3), which halves HBM traffic per core from 24 MiB
to 12 MiB — the kernel is HBM-bound, so this is ~2x. Each core streams its
shard through SBUF in [128, 8192] bf16 tiles (16 KiB per partition per
tile, the same DMA descriptor shape as the fp32 baseline's [128, 4096]),
double-buffered, raw Bass engine blocks with manual semaphores. Per tile:
ACT squares x (bf16 in/out, fp32 internal), DVE runs two
scalar_tensor_tensor passes (w_lin*x and w_quad*x^2), each with a fused
per-partition fp32 accumulate (accum_out), at 2x DVE mode for 16-bit
operands. Per-core output is a [128, 2*NT] fp32 tile of per-(tile, term)
partial sums, reduced on the host in fp64 along with the two scalar
epilogue terms (w[2W] and x[W//2] stay exact fp32).
"""

import sys
from contextlib import ExitStack

for _p in ("/opt/trn_rl_repo", "/root/.axon_site/_ro/trn_rl_repo"):
    if _p not in sys.path:
        sys.path.append(_p)

import numpy as np

W = 16777216
NCORES = 8
C = W // NCORES          # 2,097,152 elements per core per tensor
P = 128
F = 8192                 # free-dim per tile -> [128, 8192] bf16 = 2 MiB
TILE = P * F             # 1,048,576 elements
NT = C // TILE           # 2 tiles per tensor per core
NBUF = 2

_cache = {}


def _np_bf16():
    from concourse import mybir
    return mybir.dt.np(mybir.dt.bfloat16)


def _pack(inputs: dict) -> list:
    bf16 = _np_bf16()
    x = np.asarray(inputs["x"], dtype=np.float32)
    w = np.asarray(inputs["weight"], dtype=np.float32)[0]
    xs = x.astype(bf16).reshape(NCORES, NT * P, F)
    wls = w[:W].astype(bf16).reshape(NCORES, NT * P, F)
    wqs = w[W:2 * W].astype(bf16).reshape(NCORES, NT * P, F)
    return [{"x": xs[c], "wl": wls[c], "wq": wqs[c]} for c in range(NCORES)]


def _build(reps: int = 1, nbuf: int = NBUF, x2buf: int | None = None,
           f: int = F):
    import concourse.bass as bass
    from concourse import mybir

    f32 = mybir.dt.float32
    bf16 = mybir.dt.bfloat16
    nc = bass.Bass()

    if x2buf is None:
        x2buf = 2 if nbuf <= 2 else 1
    F = f
    NT = C // (P * F)

    x_d = nc.declare_dram_parameter("x", [NT * P, F], bf16, isOutput=False)
    wl_d = nc.declare_dram_parameter("wl", [NT * P, F], bf16, isOutput=False)
    wq_d = nc.declare_dram_parameter("wq", [NT * P, F], bf16, isOutput=False)
    out_d = nc.declare_dram_parameter("out", [P, 2 * NT], f32, isOutput=True)

    mult = mybir.AluOpType.mult

    with ExitStack() as ctx:
        xb = [ctx.enter_context(nc.sbuf_tensor(f"xb{s}", [P, F], bf16))
              for s in range(nbuf)]
        wlb = [ctx.enter_context(nc.sbuf_tensor(f"wlb{s}", [P, F], bf16))
               for s in range(nbuf)]
        wqb = [ctx.enter_context(nc.sbuf_tensor(f"wqb{s}", [P, F], bf16))
               for s in range(nbuf)]
        x2b = [ctx.enter_context(nc.sbuf_tensor(f"x2b{s}", [P, F], bf16))
               for s in range(x2buf)]
        prodb = ctx.enter_context(nc.sbuf_tensor("prodb", [P, F], bf16))
        accb = ctx.enter_context(nc.sbuf_tensor("accb", [P, 2 * NT], f32))

        sem_in = [ctx.enter_context(nc.semaphore(f"sem_in{s}"))
                  for s in range(nbuf)]
        sem_act = ctx.enter_context(nc.semaphore("sem_act"))
        sem_dve = ctx.enter_context(nc.semaphore("sem_dve"))
        sem_out = ctx.enter_context(nc.semaphore("sem_out"))

        with nc.Block() as block:

            G = NT * reps

            @block.sync
            def _(sync):
                for g in range(G):
                    i = g % NT
                    s = g % nbuf
                    rows = slice(i * P, (i + 1) * P)
                    if g >= nbuf:
                        # WAR: don't overwrite slot s until compute of
                        # iteration g-nbuf fully consumed it.
                        sync.wait_ge(sem_dve, 2 * (g - nbuf) + 2)
                    sync.dma_start(xb[s][:], x_d[rows, :]).then_inc(sem_in[s], 16)
                    sync.dma_start(wlb[s][:], wl_d[rows, :]).then_inc(sem_in[s], 16)
                    sync.dma_start(wqb[s][:], wq_d[rows, :]).then_inc(sem_in[s], 16)
                sync.wait_ge(sem_dve, 2 * G)
                sync.dma_start(out_d[:], accb[:]).then_inc(sem_out, 16)
                sync.wait_ge(sem_out, 16)

            @block.scalar
            def _(scalar):
                for g in range(G):
                    s = g % nbuf
                    s2 = g % x2buf
                    k = g // nbuf
                    # whole input trio for this slot landed
                    scalar.wait_ge(sem_in[s], 48 * (k + 1))
                    if g >= x2buf:
                        # WAR on x2b[s2]: quad STT of g-x2buf read it
                        scalar.wait_ge(sem_dve, 2 * (g - x2buf) + 2)
                    scalar.square(out=x2b[s2][:], in_=xb[s][:]).then_inc(sem_act, 1)

            @block.vector
            def _(vector):
                for g in range(G):
                    i = g % NT
                    s = g % nbuf
                    s2 = g % x2buf
                    k = g // nbuf
                    vector.wait_ge(sem_in[s], 48 * (k + 1))
                    vector.scalar_tensor_tensor(
                        out=prodb[:], in0=wlb[s][:], scalar=1.0, in1=xb[s][:],
                        op0=mult, op1=mult,
                        accum_out=accb[:, 2 * i:2 * i + 1],
                    ).then_inc(sem_dve, 1)
                    vector.wait_ge(sem_act, g + 1)
                    vector.scalar_tensor_tensor(
                        out=prodb[:], in0=wqb[s][:], scalar=1.0, in1=x2b[s2][:],
                        op0=mult, op1=mult,
                        accum_out=accb[:, 2 * i + 1:2 * i + 2],
                    ).then_inc(sem_dve, 1)

    return nc


def _run(inputs: dict, trace: bool = False, tmpdir: str | None = None):
    from concourse.bass_utils import run_bass_kernel_spmd

    if "nc" not in _cache:
        _cache["nc"] = _build(reps=1)
    nc = _cache["nc"]

    x = np.asarray(inputs["x"], dtype=np.float32)
    w = np.asarray(inputs["weight"], dtype=np.float32)[0]

    in_maps = _pack(inputs)
    res = run_bass_kernel_spmd(
        nc, in_maps, core_ids=list(range(NCORES)),
        trace=trace, tmpdir=tmpdir,
    )

    total = np.float64(0.0)
    for c in range(NCORES):
        total += res.results[c]["out"].astype(np.float64).sum()

    out0 = np.float32(total + np.float64(w[2 * W]))
    out1 = np.float32(x[W // 2]) - out0
    return np.stack([out0, out1]).astype(np.float32), res


def kernel(**inputs) -> np.ndarray:
    out, _ = _run(inputs)
    return out


# revision 16
# speedup vs baseline: 1.1407x; 1.1407x over previous
"""Trainium2 Bass kernel for nn_Linear_regression (quadratic regression dot).

out0 = dot(w_lin, x) + dot(w_quad, x*x) + w[2W]
out1 = x[W//2] - out0

Strategy: shard x / w_lin / w_quad along W across 8 cores. The streams are
cast to bf16 on the host during packing (the rel-err gate is 2e-2; bf16
rounding contributes ~3e-3), which halves HBM traffic per core from 24 MiB
to 12 MiB — the kernel is HBM-bound, so this is ~2x. Each core streams its
shard through SBUF in [128, 8192] bf16 tiles (16 KiB per partition per
tile, the same DMA descriptor shape as the fp32 baseline's [128, 4096]),
double-buffered, raw Bass engine blocks with manual semaphores. Per tile:
ACT squares x (bf16 in/out, fp32 internal), DVE runs two
scalar_tensor_tensor passes (w_lin*x and w_quad*x^2), each with a fused
per-partition fp32 accumulate (accum_out), at 2x DVE mode for 16-bit
operands. Per-core output is a [128, 2*NT] fp32 tile of per-(tile, term)
partial sums, reduced on the host in fp64 along with the two scalar
epilogue terms (w[2W] and x[W//2] stay exact fp32).
"""

import sys
from contextlib import ExitStack

for _p in ("/opt/trn_rl_repo", "/root/.axon_site/_ro/trn_rl_repo"):
    if _p not in sys.path:
        sys.path.append(_p)

import numpy as np

W = 16777216
NCORES = 8
C = W // NCORES          # 2,097,152 elements per core per tensor
P = 128
F = 8192                 # free-dim per tile -> [128, 8192] bf16 = 2 MiB
TILE = P * F             # 1,048,576 elements
NT = C // TILE           # 2 tiles per tensor per core
NBUF = 2

_cache = {}


def _np_bf16():
    from concourse import mybir
    return mybir.dt.np(mybir.dt.bfloat16)


def _pack(inputs: dict) -> list:
    bf16 = _np_bf16()
    x = np.asarray(inputs["x"], dtype=np.float32)
    w = np.asarray(inputs["weight"], dtype=np.float32)[0]
    xs = x.astype(bf16).reshape(NCORES, NT * P, F)
    wls = w[:W].astype(bf16).reshape(NCORES, NT * P, F)
    wqs = w[W:2 * W].astype(bf16).reshape(NCORES, NT * P, F)
    return [{"x": xs[c], "wl": wls[c], "wq": wqs[c]} for c in range(NCORES)]


def _build(reps: int = 1, nbuf: int = NBUF, x2buf: int | None = None,
           f: int = F, mode: str = "stt", acc16: bool = False):
    import concourse.bass as bass
    from concourse import mybir

    f32 = mybir.dt.float32
    bf16 = mybir.dt.bfloat16
    nc = bass.Bass()

    if x2buf is None:
        x2buf = 2 if nbuf <= 2 else 1
    F = f
    NT = C // (P * F)
    acc_dt = bf16 if acc16 else f32

    x_d = nc.declare_dram_parameter("x", [NT * P, F], bf16, isOutput=False)
    wl_d = nc.declare_dram_parameter("wl", [NT * P, F], bf16, isOutput=False)
    wq_d = nc.declare_dram_parameter("wq", [NT * P, F], bf16, isOutput=False)
    if mode == "pe":
        out_d = nc.declare_dram_parameter("out", [1, 512], f32, isOutput=True)
    else:
        out_d = nc.declare_dram_parameter("out", [P, 2 * NT], acc_dt,
                                          isOutput=True)

    mult = mybir.AluOpType.mult
    add = mybir.AluOpType.add

    with ExitStack() as ctx:
        xb = [ctx.enter_context(nc.sbuf_tensor(f"xb{s}", [P, F], bf16))
              for s in range(nbuf)]
        wlb = [ctx.enter_context(nc.sbuf_tensor(f"wlb{s}", [P, F], bf16))
               for s in range(nbuf)]
        wqb = [ctx.enter_context(nc.sbuf_tensor(f"wqb{s}", [P, F], bf16))
               for s in range(nbuf)]
        x2b = [ctx.enter_context(nc.sbuf_tensor(f"x2b{s}", [P, F], bf16))
               for s in range(x2buf)]
        if mode != "pe":
            prodb = ctx.enter_context(nc.sbuf_tensor("prodb", [P, F], bf16))
        if mode in ("split", "tsacc"):
            prodb2 = ctx.enter_context(nc.sbuf_tensor("prodb2", [P, F], bf16))
        if mode == "pe":
            prodl = [ctx.enter_context(nc.sbuf_tensor(f"prodl{s}", [P, F], bf16))
                     for s in range(2)]
            prodq = [ctx.enter_context(nc.sbuf_tensor(f"prodq{s}", [P, F], bf16))
                     for s in range(2)]
            onesb = ctx.enter_context(nc.sbuf_tensor("onesb", [P, 1], bf16))
            psum = ctx.enter_context(nc.psum_tensor("psum", [1, 512],
                                                    mybir.dt.float32))
            accps = ctx.enter_context(nc.sbuf_tensor("accps", [1, 512], f32))
        accb = ctx.enter_context(nc.sbuf_tensor("accb", [P, 2 * NT], acc_dt))

        sem_in = [ctx.enter_context(nc.semaphore(f"sem_in{s}"))
                  for s in range(nbuf)]
        sem_act = ctx.enter_context(nc.semaphore("sem_act"))
        sem_dve = ctx.enter_context(nc.semaphore("sem_dve"))
        sem_out = ctx.enter_context(nc.semaphore("sem_out"))
        if mode == "pe":
            sem_pe = ctx.enter_context(nc.semaphore("sem_pe"))

        with nc.Block() as block:

            G = NT * reps

            if mode == "dma_only":
                # Bench-only: raw DMA stream rate, no compute, no WAR
                # throttling (buffer content races are irrelevant).
                @block.sync
                def _(sync):
                    for g in range(G):
                        i = g % NT
                        s = g % nbuf
                        rows = slice(i * P, (i + 1) * P)
                        sync.dma_start(xb[s][:], x_d[rows, :]).then_inc(sem_in[s], 16)
                        sync.dma_start(wlb[s][:], wl_d[rows, :]).then_inc(sem_in[s], 16)
                        sync.dma_start(wqb[s][:], wq_d[rows, :]).then_inc(sem_in[s], 16)
                    for s in range(nbuf):
                        fills = len([g for g in range(G) if g % nbuf == s])
                        sync.wait_ge(sem_in[s], 48 * fills)
                    sync.dma_start(out_d[:], accb[:]).then_inc(sem_out, 16)
                    sync.wait_ge(sem_out, 16)

                return nc

            @block.sync
            def _(sync):
                for g in range(G):
                    i = g % NT
                    s = g % nbuf
                    rows = slice(i * P, (i + 1) * P)
                    if g >= nbuf:
                        # WAR: don't overwrite slot s until compute of
                        # iteration g-nbuf fully consumed it.
                        sync.wait_ge(sem_dve, 2 * (g - nbuf) + 2)
                    sync.dma_start(xb[s][:], x_d[rows, :]).then_inc(sem_in[s], 16)
                    sync.dma_start(wlb[s][:], wl_d[rows, :]).then_inc(sem_in[s], 16)
                    sync.dma_start(wqb[s][:], wq_d[rows, :]).then_inc(sem_in[s], 16)
                if mode == "pe":
                    sync.wait_ge(sem_dve, 2 * G + 1)
                    sync.dma_start(out_d[:], accps[:]).then_inc(sem_out, 16)
                else:
                    sync.wait_ge(sem_dve, 2 * G)
                    sync.dma_start(out_d[:], accb[:]).then_inc(sem_out, 16)
                sync.wait_ge(sem_out, 16)

            if mode != "skip_quad":
                @block.scalar
                def _(scalar):
                    for g in range(G):
                        s = g % nbuf
                        s2 = g % x2buf
                        k = g // nbuf
                        # whole input trio for this slot landed
                        scalar.wait_ge(sem_in[s], 48 * (k + 1))
                        if g >= x2buf:
                            # WAR on x2b[s2]: quad STT of g-x2buf read it
                            scalar.wait_ge(sem_dve, 2 * (g - x2buf) + 2)
                        scalar.square(out=x2b[s2][:], in_=xb[s][:]).then_inc(sem_act, 1)

            if mode == "pe":
                # DVE: plain TT products (2x bf16 mode); PE: ones-stationary
                # matmuls reduce each product tile into one accumulating
                # [1, 512] PSUM bank; DVE copies PSUM->SBUF at the end.
                NCH = F // 512
                total_mm = G * 2 * NCH

                @block.vector
                def _(vector):
                    vector.memset(onesb[:], 1.0)
                    for g in range(G):
                        s = g % nbuf
                        s2 = g % x2buf
                        k = g // nbuf
                        vector.wait_ge(sem_in[s], 48 * (k + 1))
                        if g >= 2:
                            # WAR: PE finished reading prodl[g%2] (iter g-2)
                            vector.wait_ge(sem_pe, 2 * (g - 2) + 1)
                        vector.tensor_tensor(
                            out=prodl[g % 2][:], in0=wlb[s][:], in1=xb[s][:],
                            op=mult,
                        ).then_inc(sem_dve, 1)
                        vector.wait_ge(sem_act, g + 1)
                        if g >= 2:
                            vector.wait_ge(sem_pe, 2 * (g - 2) + 2)
                        vector.tensor_tensor(
                            out=prodq[g % 2][:], in0=wqb[s][:], in1=x2b[s2][:],
                            op=mult,
                        ).then_inc(sem_dve, 1)
                    vector.wait_ge(sem_pe, 2 * G)
                    vector.tensor_copy(accps[:], psum[:]).then_inc(sem_dve, 1)

                @block.tensor
                def _(tensor):
                    n = 0
                    for g in range(G):
                        tensor.wait_ge(sem_dve, 2 * g + 1)
                        for c in range(NCH):
                            mm = tensor.matmul(
                                psum[:], onesb[:],
                                prodl[g % 2][:, 512 * c:512 * (c + 1)],
                                start=(n == 0), stop=(n == total_mm - 1),
                                skip_group_check=True,
                            )
                            n += 1
                            if c == NCH - 1:
                                mm.then_inc(sem_pe, 1)
                        tensor.wait_ge(sem_dve, 2 * g + 2)
                        for c in range(NCH):
                            mm = tensor.matmul(
                                psum[:], onesb[:],
                                prodq[g % 2][:, 512 * c:512 * (c + 1)],
                                start=(n == 0), stop=(n == total_mm - 1),
                                skip_group_check=True,
                            )
                            n += 1
                            if c == NCH - 1:
                                mm.then_inc(sem_pe, 1)

                return nc

            if mode == "split":
                # lin STT on gpsimd, quad STT on DVE (parallel engines)
                @block.gpsimd
                def _(gpsimd):
                    for g in range(G):
                        i = g % NT
                        s = g % nbuf
                        k = g // nbuf
                        gpsimd.wait_ge(sem_in[s], 48 * (k + 1))
                        gpsimd.scalar_tensor_tensor(
                            out=prodb2[:], in0=wlb[s][:], scalar=1.0,
                            in1=xb[s][:], op0=mult, op1=mult,
                            accum_out=accb[:, 2 * i:2 * i + 1],
                        ).then_inc(sem_dve, 1)

                @block.vector
                def _(vector):
                    for g in range(G):
                        i = g % NT
                        s = g % nbuf
                        s2 = g % x2buf
                        k = g // nbuf
                        vector.wait_ge(sem_in[s], 48 * (k + 1))
                        vector.wait_ge(sem_act, g + 1)
                        vector.scalar_tensor_tensor(
                            out=prodb[:], in0=wqb[s][:], scalar=1.0,
                            in1=x2b[s2][:], op0=mult, op1=mult,
                            accum_out=accb[:, 2 * i + 1:2 * i + 2],
                        ).then_inc(sem_dve, 1)

                return nc

            @block.vector
            def _(vector):
                for g in range(G):
                    i = g % NT
                    s = g % nbuf
                    s2 = g % x2buf
                    k = g // nbuf
                    vector.wait_ge(sem_in[s], 48 * (k + 1))
                    if mode == "tsacc":
                        # TT product at 2x, then single-source TS with accum
                        vector.tensor_tensor(
                            out=prodb[:], in0=wlb[s][:], in1=xb[s][:],
                            op=mult,
                        )
                        vector.tensor_scalar(
                            prodb2[:], prodb[:], 1.0, None, mult,
                            accum_out=accb[:, 2 * i:2 * i + 1],
                        ).then_inc(sem_dve, 1)
                        vector.wait_ge(sem_act, g + 1)
                        vector.tensor_tensor(
                            out=prodb[:], in0=wqb[s][:], in1=x2b[s2][:],
                            op=mult,
                        )
                        vector.tensor_scalar(
                            prodb2[:], prodb[:], 1.0, None, mult,
                            accum_out=accb[:, 2 * i + 1:2 * i + 2],
                        ).then_inc(sem_dve, 1)
                        continue
                    if mode == "tt_only":
                        # Bench-only: products without accumulate (wrong
                        # results; probes whether accum_out caps DVE at 1x)
                        vector.tensor_tensor(
                            out=prodb[:], in0=wlb[s][:], in1=xb[s][:],
                            op=mult,
                        ).then_inc(sem_dve, 1)
                        vector.wait_ge(sem_act, g + 1)
                        vector.tensor_tensor(
                            out=prodb[:], in0=wqb[s][:], in1=x2b[s2][:],
                            op=mult,
                        ).then_inc(sem_dve, 1)
                        continue
                    if mode == "skip_quad":
                        # Bench-only: single STT per tile (halved DVE load)
                        with nc.allow_low_precision(reason="bench"):
                            vector.scalar_tensor_tensor(
                                out=prodb[:], in0=wlb[s][:], scalar=1.0,
                                in1=xb[s][:], op0=mult, op1=mult,
                                accum_out=accb[:, 2 * i:2 * i + 1],
                            ).then_inc(sem_dve, 2)
                        continue
                    with nc.allow_low_precision(reason="bench acc16"):
                        if mode == "ttr":
                            vector.tensor_tensor_reduce(
                                out=prodb[:], in0=wlb[s][:], in1=xb[s][:],
                                scale=1.0, scalar=0.0, op0=mult, op1=add,
                                accum_out=accb[:, 2 * i:2 * i + 1],
                            ).then_inc(sem_dve, 1)
                        else:
                            vector.scalar_tensor_tensor(
                                out=prodb[:], in0=wlb[s][:], scalar=1.0,
                                in1=xb[s][:], op0=mult, op1=mult,
                                accum_out=accb[:, 2 * i:2 * i + 1],
                            ).then_inc(sem_dve, 1)
                        vector.wait_ge(sem_act, g + 1)
                        if mode == "ttr":
                            vector.tensor_tensor_reduce(
                                out=prodb[:], in0=wqb[s][:], in1=x2b[s2][:],
                                scale=1.0, scalar=0.0, op0=mult, op1=add,
                                accum_out=accb[:, 2 * i + 1:2 * i + 2],
                            ).then_inc(sem_dve, 1)
                        else:
                            vector.scalar_tensor_tensor(
                                out=prodb[:], in0=wqb[s][:], scalar=1.0,
                                in1=x2b[s2][:], op0=mult, op1=mult,
                                accum_out=accb[:, 2 * i + 1:2 * i + 2],
                            ).then_inc(sem_dve, 1)

    return nc


# Best measured configuration (applies to _run / the graded kernel() path)
BEST = {"nbuf": 3, "x2buf": 2}


def _run(inputs: dict, trace: bool = False, tmpdir: str | None = None):
    from concourse.bass_utils import run_bass_kernel_spmd

    if "nc" not in _cache:
        _cache["nc"] = _build(reps=1, **BEST)
    nc = _cache["nc"]

    x = np.asarray(inputs["x"], dtype=np.float32)
    w = np.asarray(inputs["weight"], dtype=np.float32)[0]

    in_maps = _pack(inputs)
    res = run_bass_kernel_spmd(
        nc, in_maps, core_ids=list(range(NCORES)),
        trace=trace, tmpdir=tmpdir,
    )

    total = np.float64(0.0)
    for c in range(NCORES):
        total += res.results[c]["out"].astype(np.float64).sum()

    out0 = np.float32(total + np.float64(w[2 * W]))
    out1 = np.float32(x[W // 2]) - out0
    return np.stack([out0, out1]).astype(np.float32), res


def kernel(**inputs) -> np.ndarray:
    out, _ = _run(inputs)
    return out


# revision 18
# speedup vs baseline: 1.1886x; 1.0420x over previous
"""Trainium2 Bass kernel for nn_Linear_regression (quadratic regression dot).

out0 = dot(w_lin, x) + dot(w_quad, x*x) + w[2W]
out1 = x[W//2] - out0

Strategy: shard x / w_lin / w_quad along W across 8 cores. The streams are
cast to bf16 on the host during packing (the rel-err gate is 2e-2; bf16
rounding contributes ~3e-3), which halves HBM traffic per core from 24 MiB
to 12 MiB — the kernel is HBM-bound, so this is ~2x. Each core streams its
shard through SBUF in [128, 8192] bf16 tiles (16 KiB per partition per
tile, the same DMA descriptor shape as the fp32 baseline's [128, 4096]),
double-buffered, raw Bass engine blocks with manual semaphores. Per tile:
ACT squares x (bf16 in/out, fp32 internal), DVE runs two
scalar_tensor_tensor passes (w_lin*x and w_quad*x^2), each with a fused
per-partition fp32 accumulate (accum_out), at 2x DVE mode for 16-bit
operands. Per-core output is a [128, 2*NT] fp32 tile of per-(tile, term)
partial sums, reduced on the host in fp64 along with the two scalar
epilogue terms (w[2W] and x[W//2] stay exact fp32).
"""

import sys
from contextlib import ExitStack

for _p in ("/opt/trn_rl_repo", "/root/.axon_site/_ro/trn_rl_repo"):
    if _p not in sys.path:
        sys.path.append(_p)

import numpy as np

W = 16777216
NCORES = 8
C = W // NCORES          # 2,097,152 elements per core per tensor
P = 128
F = 8192                 # free-dim per tile -> [128, 8192] bf16 = 2 MiB
TILE = P * F             # 1,048,576 elements
NT = C // TILE           # 2 tiles per tensor per core
NBUF = 2

_cache = {}


def _np_bf16():
    from concourse import mybir
    return mybir.dt.np(mybir.dt.bfloat16)


def _pack(inputs: dict) -> list:
    bf16 = _np_bf16()
    x = np.asarray(inputs["x"], dtype=np.float32)
    w = np.asarray(inputs["weight"], dtype=np.float32)[0]
    xs = x.astype(bf16).reshape(NCORES, NT * P, F)
    wls = w[:W].astype(bf16).reshape(NCORES, NT * P, F)
    wqs = w[W:2 * W].astype(bf16).reshape(NCORES, NT * P, F)
    return [{"x": xs[c], "wl": wls[c], "wq": wqs[c]} for c in range(NCORES)]


def _build(reps: int = 1, nbuf: int = NBUF, x2buf: int | None = None,
           f: int = F, mode: str = "stt", acc16: bool = False):
    import concourse.bass as bass
    from concourse import mybir

    f32 = mybir.dt.float32
    bf16 = mybir.dt.bfloat16
    nc = bass.Bass()

    if x2buf is None:
        x2buf = 2 if nbuf <= 2 else 1
    F = f
    NT = C // (P * F)
    acc_dt = bf16 if acc16 else f32

    x_d = nc.declare_dram_parameter("x", [NT * P, F], bf16, isOutput=False)
    wl_d = nc.declare_dram_parameter("wl", [NT * P, F], bf16, isOutput=False)
    wq_d = nc.declare_dram_parameter("wq", [NT * P, F], bf16, isOutput=False)
    if mode == "pe":
        out_d = nc.declare_dram_parameter("out", [1, 512], f32, isOutput=True)
    else:
        out_d = nc.declare_dram_parameter("out", [P, 2 * NT], acc_dt,
                                          isOutput=True)

    mult = mybir.AluOpType.mult
    add = mybir.AluOpType.add

    with ExitStack() as ctx:
        xb = [ctx.enter_context(nc.sbuf_tensor(f"xb{s}", [P, F], bf16))
              for s in range(nbuf)]
        wlb = [ctx.enter_context(nc.sbuf_tensor(f"wlb{s}", [P, F], bf16))
               for s in range(nbuf)]
        wqb = [ctx.enter_context(nc.sbuf_tensor(f"wqb{s}", [P, F], bf16))
               for s in range(nbuf)]
        x2b = [ctx.enter_context(nc.sbuf_tensor(f"x2b{s}", [P, F], bf16))
               for s in range(x2buf)]
        if mode != "pe":
            prodb = ctx.enter_context(nc.sbuf_tensor("prodb", [P, F], bf16))
        if mode in ("split", "tsacc"):
            prodb2 = ctx.enter_context(nc.sbuf_tensor("prodb2", [P, F], bf16))
        if mode == "pe":
            prodl = [ctx.enter_context(nc.sbuf_tensor(f"prodl{s}", [P, F], bf16))
                     for s in range(2)]
            prodq = [ctx.enter_context(nc.sbuf_tensor(f"prodq{s}", [P, F], bf16))
                     for s in range(2)]
            onesb = ctx.enter_context(nc.sbuf_tensor("onesb", [P, 1], bf16))
            psum = ctx.enter_context(nc.psum_tensor("psum", [1, 512],
                                                    mybir.dt.float32))
            accps = ctx.enter_context(nc.sbuf_tensor("accps", [1, 512], f32))
        accb = ctx.enter_context(nc.sbuf_tensor("accb", [P, 2 * NT], acc_dt))

        sem_in = [ctx.enter_context(nc.semaphore(f"sem_in{s}"))
                  for s in range(nbuf)]
        sem_act = ctx.enter_context(nc.semaphore("sem_act"))
        sem_dve = ctx.enter_context(nc.semaphore("sem_dve"))
        sem_out = ctx.enter_context(nc.semaphore("sem_out"))
        if mode == "pe":
            sem_pe = ctx.enter_context(nc.semaphore("sem_pe"))

        with nc.Block() as block:

            G = NT * reps

            if mode == "dma_only":
                # Bench-only: raw DMA stream rate, no compute, no WAR
                # throttling (buffer content races are irrelevant).
                @block.sync
                def _(sync):
                    for g in range(G):
                        i = g % NT
                        s = g % nbuf
                        rows = slice(i * P, (i + 1) * P)
                        sync.dma_start(xb[s][:], x_d[rows, :]).then_inc(sem_in[s], 16)
                        sync.dma_start(wlb[s][:], wl_d[rows, :]).then_inc(sem_in[s], 16)
                        sync.dma_start(wqb[s][:], wq_d[rows, :]).then_inc(sem_in[s], 16)
                    for s in range(nbuf):
                        fills = len([g for g in range(G) if g % nbuf == s])
                        sync.wait_ge(sem_in[s], 48 * fills)
                    sync.dma_start(out_d[:], accb[:]).then_inc(sem_out, 16)
                    sync.wait_ge(sem_out, 16)

                return nc

            @block.sync
            def _(sync):
                for g in range(G):
                    i = g % NT
                    s = g % nbuf
                    rows = slice(i * P, (i + 1) * P)
                    if g >= nbuf:
                        # WAR: don't overwrite slot s until compute of
                        # iteration g-nbuf fully consumed it.
                        sync.wait_ge(sem_dve, 2 * (g - nbuf) + 2)
                    sync.dma_start(xb[s][:], x_d[rows, :]).then_inc(sem_in[s], 16)
                    sync.dma_start(wlb[s][:], wl_d[rows, :]).then_inc(sem_in[s], 16)
                    sync.dma_start(wqb[s][:], wq_d[rows, :]).then_inc(sem_in[s], 16)
                if mode == "pe":
                    sync.wait_ge(sem_dve, 2 * G + 1)
                    sync.dma_start(out_d[:], accps[:]).then_inc(sem_out, 16)
                else:
                    sync.wait_ge(sem_dve, 2 * G)
                    sync.dma_start(out_d[:], accb[:]).then_inc(sem_out, 16)
                sync.wait_ge(sem_out, 16)

            if mode != "skip_quad":
                @block.scalar
                def _(scalar):
                    for g in range(G):
                        s = g % nbuf
                        s2 = g % x2buf
                        k = g // nbuf
                        # whole input trio for this slot landed
                        scalar.wait_ge(sem_in[s], 48 * (k + 1))
                        if g >= x2buf:
                            # WAR on x2b[s2]: quad STT of g-x2buf read it
                            scalar.wait_ge(sem_dve, 2 * (g - x2buf) + 2)
                        scalar.square(out=x2b[s2][:], in_=xb[s][:]).then_inc(sem_act, 1)

            if mode == "pe":
                # DVE: plain TT products (2x bf16 mode); PE: ones-stationary
                # matmuls reduce each product tile into one accumulating
                # [1, 512] PSUM bank; DVE copies PSUM->SBUF at the end.
                NCH = F // 512
                total_mm = G * 2 * NCH

                @block.vector
                def _(vector):
                    vector.memset(onesb[:], 1.0)
                    for g in range(G):
                        s = g % nbuf
                        s2 = g % x2buf
                        k = g // nbuf
                        vector.wait_ge(sem_in[s], 48 * (k + 1))
                        if g >= 2:
                            # WAR: PE finished reading prodl[g%2] (iter g-2)
                            vector.wait_ge(sem_pe, 2 * (g - 2) + 1)
                        vector.tensor_tensor(
                            out=prodl[g % 2][:], in0=wlb[s][:], in1=xb[s][:],
                            op=mult,
                        ).then_inc(sem_dve, 1)
                        vector.wait_ge(sem_act, g + 1)
                        if g >= 2:
                            vector.wait_ge(sem_pe, 2 * (g - 2) + 2)
                        vector.tensor_tensor(
                            out=prodq[g % 2][:], in0=wqb[s][:], in1=x2b[s2][:],
                            op=mult,
                        ).then_inc(sem_dve, 1)
                    vector.wait_ge(sem_pe, 2 * G)
                    vector.tensor_copy(accps[:], psum[:]).then_inc(sem_dve, 1)

                @block.tensor
                def _(tensor):
                    n = 0
                    for g in range(G):
                        tensor.wait_ge(sem_dve, 2 * g + 1)
                        for c in range(NCH):
                            mm = tensor.matmul(
                                psum[:], onesb[:],
                                prodl[g % 2][:, 512 * c:512 * (c + 1)],
                                start=(n == 0), stop=(n == total_mm - 1),
                                skip_group_check=True,
                            )
                            n += 1
                            if c == NCH - 1:
                                mm.then_inc(sem_pe, 1)
                        tensor.wait_ge(sem_dve, 2 * g + 2)
                        for c in range(NCH):
                            mm = tensor.matmul(
                                psum[:], onesb[:],
                                prodq[g % 2][:, 512 * c:512 * (c + 1)],
                                start=(n == 0), stop=(n == total_mm - 1),
                                skip_group_check=True,
                            )
                            n += 1
                            if c == NCH - 1:
                                mm.then_inc(sem_pe, 1)

                return nc

            if mode == "split":
                # lin STT on gpsimd, quad STT on DVE (parallel engines)
                @block.gpsimd
                def _(gpsimd):
                    for g in range(G):
                        i = g % NT
                        s = g % nbuf
                        k = g // nbuf
                        gpsimd.wait_ge(sem_in[s], 48 * (k + 1))
                        gpsimd.scalar_tensor_tensor(
                            out=prodb2[:], in0=wlb[s][:], scalar=1.0,
                            in1=xb[s][:], op0=mult, op1=mult,
                            accum_out=accb[:, 2 * i:2 * i + 1],
                        ).then_inc(sem_dve, 1)

                @block.vector
                def _(vector):
                    for g in range(G):
                        i = g % NT
                        s = g % nbuf
                        s2 = g % x2buf
                        k = g // nbuf
                        vector.wait_ge(sem_in[s], 48 * (k + 1))
                        vector.wait_ge(sem_act, g + 1)
                        vector.scalar_tensor_tensor(
                            out=prodb[:], in0=wqb[s][:], scalar=1.0,
                            in1=x2b[s2][:], op0=mult, op1=mult,
                            accum_out=accb[:, 2 * i + 1:2 * i + 2],
                        ).then_inc(sem_dve, 1)

                return nc

            @block.vector
            def _(vector):
                for g in range(G):
                    i = g % NT
                    s = g % nbuf
                    s2 = g % x2buf
                    k = g // nbuf
                    vector.wait_ge(sem_in[s], 48 * (k + 1))
                    if mode == "tsacc":
                        # TT product at 2x, then single-source TS with accum
                        vector.tensor_tensor(
                            out=prodb[:], in0=wlb[s][:], in1=xb[s][:],
                            op=mult,
                        )
                        vector.tensor_scalar(
                            prodb2[:], prodb[:], 1.0, None, mult,
                            accum_out=accb[:, 2 * i:2 * i + 1],
                        ).then_inc(sem_dve, 1)
                        vector.wait_ge(sem_act, g + 1)
                        vector.tensor_tensor(
                            out=prodb[:], in0=wqb[s][:], in1=x2b[s2][:],
                            op=mult,
                        )
                        vector.tensor_scalar(
                            prodb2[:], prodb[:], 1.0, None, mult,
                            accum_out=accb[:, 2 * i + 1:2 * i + 2],
                        ).then_inc(sem_dve, 1)
                        continue
                    if mode == "tt_only":
                        # Bench-only: products without accumulate (wrong
                        # results; probes whether accum_out caps DVE at 1x)
                        vector.tensor_tensor(
                            out=prodb[:], in0=wlb[s][:], in1=xb[s][:],
                            op=mult,
                        ).then_inc(sem_dve, 1)
                        vector.wait_ge(sem_act, g + 1)
                        vector.tensor_tensor(
                            out=prodb[:], in0=wqb[s][:], in1=x2b[s2][:],
                            op=mult,
                        ).then_inc(sem_dve, 1)
                        continue
                    if mode == "skip_quad":
                        # Bench-only: single STT per tile (halved DVE load)
                        with nc.allow_low_precision(reason="bench"):
                            vector.scalar_tensor_tensor(
                                out=prodb[:], in0=wlb[s][:], scalar=1.0,
                                in1=xb[s][:], op0=mult, op1=mult,
                                accum_out=accb[:, 2 * i:2 * i + 1],
                            ).then_inc(sem_dve, 2)
                        continue
                    with nc.allow_low_precision(reason="bench acc16"):
                        if mode == "ttr":
                            vector.tensor_tensor_reduce(
                                out=prodb[:], in0=wlb[s][:], in1=xb[s][:],
                                scale=1.0, scalar=0.0, op0=mult, op1=add,
                                accum_out=accb[:, 2 * i:2 * i + 1],
                            ).then_inc(sem_dve, 1)
                        else:
                            vector.scalar_tensor_tensor(
                                out=prodb[:], in0=wlb[s][:], scalar=1.0,
                                in1=xb[s][:], op0=mult, op1=mult,
                                accum_out=accb[:, 2 * i:2 * i + 1],
                            ).then_inc(sem_dve, 1)
                        vector.wait_ge(sem_act, g + 1)
                        if mode == "ttr":
                            vector.tensor_tensor_reduce(
                                out=prodb[:], in0=wqb[s][:], in1=x2b[s2][:],
                                scale=1.0, scalar=0.0, op0=mult, op1=add,
                                accum_out=accb[:, 2 * i + 1:2 * i + 2],
                            ).then_inc(sem_dve, 1)
                        else:
                            vector.scalar_tensor_tensor(
                                out=prodb[:], in0=wqb[s][:], scalar=1.0,
                                in1=x2b[s2][:], op0=mult, op1=mult,
                                accum_out=accb[:, 2 * i + 1:2 * i + 2],
                            ).then_inc(sem_dve, 1)

    return nc


# Best measured configuration (applies to _run / the graded kernel() path)
BEST = {"mode": "pe", "f": 4096, "nbuf": 6, "x2buf": 3}


def _run(inputs: dict, trace: bool = False, tmpdir: str | None = None):
    from concourse.bass_utils import run_bass_kernel_spmd

    if "nc" not in _cache:
        _cache["nc"] = _build(reps=1, **BEST)
    nc = _cache["nc"]

    x = np.asarray(inputs["x"], dtype=np.float32)
    w = np.asarray(inputs["weight"], dtype=np.float32)[0]

    fb = BEST.get("f", F)
    in_maps = [{k: v.reshape(C // fb, fb) for k, v in m.items()}
               for m in _pack(inputs)]
    res = run_bass_kernel_spmd(
        nc, in_maps, core_ids=list(range(NCORES)),
        trace=trace, tmpdir=tmpdir,
    )

    total = np.float64(0.0)
    for c in range(NCORES):
        total += res.results[c]["out"].astype(np.float64).sum()

    out0 = np.float32(total + np.float64(w[2 * W]))
    out1 = np.float32(x[W // 2]) - out0
    return np.stack([out0, out1]).astype(np.float32), res


def kernel(**inputs) -> np.ndarray:
    out, _ = _run(inputs)
    return out
